# revision 8
# baseline (speedup 1.0000x reference)
"""Trainium2 Bass kernel for NodeTimeSeriesDecoder (per-node 2-layer LSTM over T=256).

Sharding: data-parallel over graphs across 8 cores (whole graphs -> contiguous
node blocks). Per core nodes are padded per-graph to multiples of B=512 and
processed as independent per-block LSTM chains; NI chains are interleaved in
one time loop so all engines stay busy despite the per-chain serial dependency.
Per-(graph, t) encoder/bias tables are precomputed on the host (tiny). The
decoder emits node-major output via transposed matmuls so DRAM writes are
contiguous per node.
"""
import sys
sys.path.insert(0, "/opt/trn_rl_repo")
import numpy as np
import ml_dtypes
import concourse.bass as bass
import concourse.bacc as bacc
import concourse.tile as tile
from concourse import mybir
from concourse.bass_utils import run_bass_kernel_spmd

F32 = mybir.dt.float32
F32R = mybir.dt.float32r
BF16 = mybir.dt.bfloat16
AF = mybir.ActivationFunctionType
DS = bass.DynSlice

H = 128
T = 256
GM = 3
ND = 6
B = 512
NCORES = 8
NI = 4        # interleaved independent block chains
UNROLL = 4
THALF = T // 2

GATE_FUNCS = [AF.Sigmoid, AF.Sigmoid, AF.Tanh, AF.Sigmoid]  # i, f, g, o


def build_nc(NBLK, NTAB, blkmap):
    NPAD = NBLK * B
    nc = bacc.Bacc(None, target_bir_lowering=False)

    node_t_ext = nc.declare_dram_parameter("node_t", [ND, NPAD], F32R, isOutput=False)
    m12_ext = nc.declare_dram_parameter("m12", [H, NTAB, 8, T], BF16, isOutput=False)
    e0_ext = nc.declare_dram_parameter("e0", [H, NTAB], F32, isOutput=False)
    wencT_node_ext = nc.declare_dram_parameter("wencT_node", [ND, H], F32R, isOutput=False)
    wihT0_ext = nc.declare_dram_parameter("wihT0", [H, 4 * H], F32R, isOutput=False)
    whhT0_ext = nc.declare_dram_parameter("whhT0", [H, 4 * H], F32R, isOutput=False)
    wihT1_ext = nc.declare_dram_parameter("wihT1", [H, 4 * H], F32R, isOutput=False)
    whhT1_ext = nc.declare_dram_parameter("whhT1", [H, 4 * H], F32R, isOutput=False)
    wd1T_ext = nc.declare_dram_parameter("wd1T", [H, 2, 64], F32R, isOutput=False)
    wd2T_ext = nc.declare_dram_parameter("wd2T", [64, 4], F32R, isOutput=False)
    bd1_ext = nc.declare_dram_parameter("bd1", [64, 1], F32, isOutput=False)
    bd2rep_ext = nc.declare_dram_parameter("bd2rep", [H, GM], F32, isOutput=False)
    y_ext = nc.declare_dram_parameter("y", [NPAD, T, GM], F32, isOutput=True)

    with tile.TileContext(nc) as tc:
        with tc.tile_pool(name="consts", bufs=1) as cp, \
             tc.tile_pool(name="work", bufs=1) as wp, \
             tc.tile_pool(name="ps", bufs=8, space="PSUM") as pp:

            wencT_node = cp.tile([ND, H], F32R)
            wihT0 = cp.tile([H, 4 * H], F32R)
            whhT0 = cp.tile([H, 4 * H], F32R)
            wihT1 = cp.tile([H, 4 * H], F32R)
            whhT1 = cp.tile([H, 4 * H], F32R)
            wd1T = cp.tile([H, 2, 64], F32R)
            wd2T = cp.tile([64, 4], F32R)
            bd1 = cp.tile([64, 1], F32)
            bd2rep = cp.tile([H, GM], F32)
            m12 = cp.tile([H, NTAB, 8, T], BF16)
            e0 = cp.tile([H, NTAB], F32)
            for dst, src in [(wencT_node, wencT_node_ext), (wihT0, wihT0_ext),
                             (whhT0, whhT0_ext), (wihT1, wihT1_ext), (whhT1, whhT1_ext),
                             (wd1T, wd1T_ext), (wd2T, wd2T_ext), (bd1, bd1_ext),
                             (bd2rep, bd2rep_ext), (m12, m12_ext), (e0, e0_ext)]:
                nc.sync.dma_start(out=dst, in_=src[:])

            for bg in range(0, NBLK, NI):
                blocks = list(range(bg, min(bg + NI, NBLK)))
                ctx = {}
                for i, b in enumerate(blocks):
                    nb = wp.tile([ND, B], F32R, tag="nb", bufs=2, name=f"nb{b}")
                    nc.sync.dma_start(out=nb, in_=node_t_ext[:, b * B:(b + 1) * B])
                    px = pp.tile([H, B], F32, tag="ps", name=f"px{b}")
                    nc.tensor.matmul(px, wencT_node, nb, start=True, stop=True)
                    enc = wp.tile([H, B], F32R, tag=f"encn{i}", bufs=1, name=f"encn{b}")
                    nc.vector.tensor_copy(enc, px)
                    h1 = [wp.tile([H, B], F32R, tag=f"h1_{i}{p}", bufs=1, name=f"h1_{i}{p}_{b}") for p in range(2)]
                    c1 = [wp.tile([H, B], F32, tag=f"c1_{i}{p}", bufs=1, name=f"c1_{i}{p}_{b}") for p in range(2)]
                    h2 = [wp.tile([H, B], F32R, tag=f"h2_{i}{p}", bufs=1, name=f"h2_{i}{p}_{b}") for p in range(2)]
                    c2 = [wp.tile([H, B], F32R, tag=f"c2_{i}{p}", bufs=1, name=f"c2_{i}{p}_{b}") for p in range(2)]
                    for dst in (h1[0], c1[0], h2[0], c2[0]):
                        nc.vector.tensor_scalar_add(dst, px, e0[:, blkmap[b]:blkmap[b] + 1])
                    yacc = wp.tile([H, 4, THALF, GM], F32, tag=f"yacc{i}", bufs=1, name=f"yacc{b}")
                    stg = [wp.tile([H, 8], F32, tag=f"stg{i}{p}", bufs=1, name=f"stg{i}{p}_{b}")
                           for p in range(2)]
                    ctx[i] = dict(b=b, g=blkmap[b], enc=enc, h1=h1, c1=c1, h2=h2, c2=c2,
                                  yacc=yacc, stg=stg)

                def timestep(i, t, th, k):
                    g = ctx[i]["g"]
                    enc, yacc, stg = ctx[i]["enc"], ctx[i]["yacc"], ctx[i]["stg"]
                    h1, c1, h2, c2 = ctx[i]["h1"], ctx[i]["c1"], ctx[i]["h2"], ctx[i]["c2"]
                    pcur, pnxt = k % 2, 1 - (k % 2)
                    nc.vector.tensor_copy(stg[pcur], m12[:, g, :, DS(t, 1)].squeeze(-1))

                    acts1 = []
                    for c in range(4):
                        pg = pp.tile([H, B], F32, tag="ps", name=f"g1_{c}")
                        nc.tensor.matmul(pg, whhT0[:, c * H:(c + 1) * H], h1[pcur],
                                         start=True, stop=False)
                        nc.tensor.matmul(pg, wihT0[:, c * H:(c + 1) * H], enc,
                                         start=False, stop=True)
                        a = wp.tile([H, B], F32, tag="acts", bufs=14, name=f"a1_{c}")
                        nc.scalar.activation(out=a, in_=pg, func=GATE_FUNCS[c],
                                             bias=stg[pcur][:, c:c + 1], scale=1.0)
                        acts1.append(a)
                    t1 = wp.tile([H, B], F32, tag="prod", bufs=8, name="t1")
                    t2 = wp.tile([H, B], F32, tag="prod", bufs=8, name="t2")
                    nc.vector.tensor_mul(t1, acts1[0], acts1[2])
                    nc.vector.tensor_mul(t2, acts1[1], c1[pcur])
                    nc.vector.tensor_add(c1[pnxt], t1, t2)
                    tc1 = wp.tile([H, B], F32, tag="acts", bufs=14, name="tc1")
                    nc.scalar.activation(out=tc1, in_=c1[pnxt], func=AF.Tanh)
                    nc.vector.tensor_mul(h1[pnxt], acts1[3], tc1)

                    acts2 = []
                    for c in range(4):
                        pg2 = pp.tile([H, B], F32, tag="ps", name=f"g2_{c}")
                        nc.tensor.matmul(pg2, wihT1[0:H - GM, c * H:(c + 1) * H],
                                         h1[pnxt][0:H - GM, :], start=True, stop=False)
                        nc.tensor.matmul(pg2, whhT1[:, c * H:(c + 1) * H], h2[pcur],
                                         start=False, stop=True)
                        a2 = wp.tile([H, B], F32, tag="acts", bufs=14, name=f"a2_{c}")
                        nc.scalar.activation(out=a2, in_=pg2, func=GATE_FUNCS[c],
                                             bias=stg[pcur][:, 4 + c:5 + c], scale=1.0)
                        acts2.append(a2)
                    t3 = wp.tile([H, B], F32, tag="prod", bufs=8, name="t3")
                    t4 = wp.tile([H, B], F32, tag="prod", bufs=8, name="t4")
                    nc.vector.tensor_mul(t3, acts2[0], acts2[2])
                    nc.vector.tensor_mul(t4, acts2[1], c2[pcur])
                    nc.vector.tensor_add(c2[pnxt], t3, t4)
                    tc2 = wp.tile([H, B], F32, tag="acts", bufs=14, name="tc2")
                    nc.scalar.activation(out=tc2, in_=c2[pnxt], func=AF.Tanh)
                    nc.vector.tensor_mul(h2[pnxt], acts2[3], tc2)

                    pd = pp.tile([64, B], F32, tag="ps", name="pd")
                    nc.tensor.matmul(pd, wd1T[:, 0, :], h2[pnxt], start=True, stop=False)
                    nc.tensor.matmul(pd, wd1T[:, 1, :], c2[pnxt], start=False, stop=True)
                    relu = wp.tile([64, B], F32R, tag="relu", bufs=4, name="relu")
                    nc.scalar.activation(out=relu, in_=pd, func=AF.Relu, bias=bd1, scale=1.0)
                    py = pp.tile([H, 16], F32, tag="ps", name="py")
                    for j in range(4):
                        nc.tensor.matmul(py[:, j * 4:(j + 1) * 4],
                                         relu[:, j * H:(j + 1) * H], wd2T,
                                         start=True, stop=True)
                    nc.vector.tensor_add(
                        yacc[:, :, DS(th, 1), :].squeeze(2),
                        py.rearrange("p (j four) -> p j four", four=4)[:, :, 0:GM],
                        bd2rep.unsqueeze(1).broadcast_to([H, 4, GM]))

                for half in range(2):
                    with tc.For_i(0, THALF, UNROLL) as tv:
                        for k in range(UNROLL):
                            for i in range(len(blocks)):
                                timestep(i, tv + k + half * THALF, tv + k, k)
                    for i, b in enumerate(blocks):
                        for j in range(4):
                            nc.sync.dma_start(
                                out=y_ext[b * B + j * H:b * B + (j + 1) * H,
                                          half * THALF:(half + 1) * THALF, :],
                                in_=ctx[i]["yacc"][:, j])

    nc.finalize()
    return nc


_CACHE = {}
_LAST_IN_MAPS = None


def _get_nc(NBLK, NTAB, blkmap):
    key = (NBLK, NTAB, tuple(blkmap))
    if key not in _CACHE:
        _CACHE[key] = build_nc(NBLK, NTAB, blkmap)
    return _CACHE[key]


def kernel(node, ptr, graph_time_series_behavior, ground_motions,
           W_enc, b_enc, W_ih, W_hh, b_ih, b_hh, W_d1, b_d1, W_d2, b_d2):
    node = np.asarray(node, np.float32)
    ptr = np.asarray(ptr, np.int64)
    lat = np.asarray(graph_time_series_behavior, np.float32)
    gms = np.asarray(ground_motions, np.float32)
    W_enc = np.asarray(W_enc, np.float32); b_enc_a = np.asarray(b_enc, np.float32)
    W_ih = np.asarray(W_ih, np.float32); W_hh = np.asarray(W_hh, np.float32)
    b_ih = np.asarray(b_ih, np.float32); b_hh = np.asarray(b_hh, np.float32)
    W_d1 = np.asarray(W_d1, np.float32); b_d1_a = np.asarray(b_d1, np.float32)
    W_d2 = np.asarray(W_d2, np.float32); b_d2_a = np.asarray(b_d2, np.float32)

    N = node.shape[0]
    BS = lat.shape[0]
    gsizes = np.diff(ptr).astype(np.int64)
    assert gsizes.sum() == N

    gper = (BS + NCORES - 1) // NCORES
    core_graphs = [list(range(c * gper, min((c + 1) * gper, BS))) for c in range(NCORES)]
    NTAB = max(len(cg) for cg in core_graphs)
    core_blkmaps, core_nblk = [], []
    for cg in core_graphs:
        bm = []
        for slot, g in enumerate(cg):
            bm += [slot] * int((gsizes[g] + B - 1) // B)
        core_blkmaps.append(bm)
        core_nblk.append(len(bm))
    NBLK = max(core_nblk) if max(core_nblk) > 0 else 1
    for bm in core_blkmaps:
        bm += [0] * (NBLK - len(bm))
    if all(bm == core_blkmaps[0] for bm in core_blkmaps):
        blkmap = core_blkmaps[0]
        per_block_tabs = False
    else:
        blkmap = list(range(NBLK))
        NTAB = NBLK
        per_block_tabs = True

    NPAD = NBLK * B

    # host-precomputed per-(graph, t) tables (small)
    enc_mix = np.einsum("hk,gtk->gth", W_enc[:, ND:ND + H], lat) \
        + np.einsum("hk,gtk->gth", W_enc[:, ND + H:], gms) + b_enc_a
    m1 = np.einsum("rh,gth->gtr", W_ih[0], enc_mix) + (b_ih[0] + b_hh[0])
    m2 = np.einsum("rk,gtk->gtr", W_ih[1][:, H - GM:], gms) + (b_ih[1] + b_hh[1])
    m12_full = np.concatenate([m1.reshape(BS, T, 4, H), m2.reshape(BS, T, 4, H)], axis=2)
    m12_full = np.ascontiguousarray(m12_full.transpose(3, 0, 2, 1))  # [H, BS, 8, T]
    e0_full = np.ascontiguousarray(enc_mix[:, 0, :].T)               # [H, BS]

    weights_common = dict(
        wencT_node=np.ascontiguousarray(W_enc[:, :ND].T),
        wihT0=np.ascontiguousarray(W_ih[0].T),
        whhT0=np.ascontiguousarray(W_hh[0].T),
        wihT1=np.ascontiguousarray(W_ih[1].T),
        whhT1=np.ascontiguousarray(W_hh[1].T),
        wd1T=np.ascontiguousarray(np.stack([W_d1[:, :H].T, W_d1[:, H:].T], axis=1)),
        wd2T=np.ascontiguousarray(np.concatenate([W_d2.T, np.zeros((64, 1), np.float32)], 1)),
        bd1=b_d1_a.reshape(64, 1),
        bd2rep=np.ascontiguousarray(np.broadcast_to(b_d2_a, (H, GM))),
    )

    in_maps, core_index_maps = [], []
    for c, cg in enumerate(core_graphs):
        node_pad = np.zeros((NPAD, ND), np.float32)
        idx_map = np.full(NPAD, -1, np.int64)
        pos = 0
        for g in cg:
            s, e = int(ptr[g]), int(ptr[g + 1])
            n = e - s
            node_pad[pos:pos + n] = node[s:e]
            idx_map[pos:pos + n] = np.arange(s, e)
            pos += int((n + B - 1) // B) * B
        m12_c = np.zeros((H, NTAB, 8, T), np.float32)
        e0_c = np.zeros((H, NTAB), np.float32)
        if per_block_tabs:
            bi = 0
            for g in cg:
                for _ in range(int((gsizes[g] + B - 1) // B)):
                    m12_c[:, bi] = m12_full[:, g]
                    e0_c[:, bi] = e0_full[:, g]
                    bi += 1
        else:
            for slot, g in enumerate(cg):
                m12_c[:, slot] = m12_full[:, g]
                e0_c[:, slot] = e0_full[:, g]
        in_maps.append(dict(
            node_t=np.ascontiguousarray(node_pad.T),
            m12=m12_c.astype(ml_dtypes.bfloat16),
            e0=e0_c,
            **weights_common,
        ))
        core_index_maps.append(idx_map)

    global _LAST_IN_MAPS
    _LAST_IN_MAPS = in_maps
    nc = _get_nc(NBLK, NTAB, blkmap)
    res = run_bass_kernel_spmd(nc, in_maps, list(range(NCORES)))

    out = np.empty((N, T, GM), np.float32)
    for c in range(NCORES):
        y = res.results[c]["y"]
        m = core_index_maps[c]
        valid = m >= 0
        out[m[valid]] = y[valid]
    return out


# revision 9
# speedup vs baseline: 2.7864x; 2.7864x over previous
"""Trainium2 Bass kernel for NodeTimeSeriesDecoder (per-node 2-layer LSTM over T=256).

Sharding: data-parallel over graphs across 8 cores (whole graphs -> contiguous
node blocks). Per core nodes are padded per-graph to multiples of B=512 and
processed as independent per-block LSTM chains; NI chains are interleaved in
one time loop so all engines stay busy despite the per-chain serial dependency.
Per-(graph, t) encoder/bias tables are precomputed on the host (tiny). The
decoder emits node-major output via transposed matmuls so DRAM writes are
contiguous per node.
"""
import sys
sys.path.insert(0, "/opt/trn_rl_repo")
import numpy as np
import ml_dtypes
import concourse.bass as bass
import concourse.bacc as bacc
import concourse.tile as tile
from concourse import mybir
from concourse.bass_utils import run_bass_kernel_spmd

F32 = mybir.dt.float32
F32R = mybir.dt.float32r
BF16 = mybir.dt.bfloat16
AF = mybir.ActivationFunctionType
DS = bass.DynSlice

H = 128
T = 256
GM = 3
ND = 6
B = 1024
NCORES = 8
NI = 4        # interleaved independent block chains
UNROLL = 4
THALF = T // 2

GATE_FUNCS = [AF.Sigmoid, AF.Sigmoid, AF.Tanh, AF.Sigmoid]  # i, f, g, o


def build_nc(NBLK, NTAB, blkmap):
    NPAD = NBLK * B
    nc = bacc.Bacc(None, target_bir_lowering=False)

    node_t_ext = nc.declare_dram_parameter("node_t", [ND, NPAD], F32R, isOutput=False)
    m12_ext = nc.declare_dram_parameter("m12", [H, NTAB, 8, T], BF16, isOutput=False)
    e0_ext = nc.declare_dram_parameter("e0", [H, NTAB], F32, isOutput=False)
    wencT_node_ext = nc.declare_dram_parameter("wencT_node", [ND, H], F32R, isOutput=False)
    wihT0_ext = nc.declare_dram_parameter("wihT0", [H, 4 * H], F32R, isOutput=False)
    whhT0_ext = nc.declare_dram_parameter("whhT0", [H, 4 * H], F32R, isOutput=False)
    wihT1_ext = nc.declare_dram_parameter("wihT1", [H, 4 * H], F32R, isOutput=False)
    whhT1_ext = nc.declare_dram_parameter("whhT1", [H, 4 * H], F32R, isOutput=False)
    wd1T_ext = nc.declare_dram_parameter("wd1T", [H, 2, 64], F32R, isOutput=False)
    wd2T_ext = nc.declare_dram_parameter("wd2T", [64, 4], F32R, isOutput=False)
    bd1_ext = nc.declare_dram_parameter("bd1", [64, 1], F32, isOutput=False)
    bd2rep_ext = nc.declare_dram_parameter("bd2rep", [H, GM], F32, isOutput=False)
    y_ext = nc.declare_dram_parameter("y", [NPAD, T, GM], F32, isOutput=True)

    with tile.TileContext(nc) as tc:
        with tc.tile_pool(name="consts", bufs=1) as cp, \
             tc.tile_pool(name="work", bufs=1) as wp, \
             tc.tile_pool(name="ps", bufs=8, space="PSUM") as pp:

            wencT_node = cp.tile([ND, H], F32R)
            wihT0 = cp.tile([H, 4 * H], F32R)
            whhT0 = cp.tile([H, 4 * H], F32R)
            wihT1 = cp.tile([H, 4 * H], F32R)
            whhT1 = cp.tile([H, 4 * H], F32R)
            wd1T = cp.tile([H, 2, 64], F32R)
            wd2T = cp.tile([64, 4], F32R)
            bd1 = cp.tile([64, 1], F32)
            bd2rep = cp.tile([H, GM], F32)
            m12 = cp.tile([H, NTAB, 8, T], BF16)
            e0 = cp.tile([H, NTAB], F32)
            for dst, src in [(wencT_node, wencT_node_ext), (wihT0, wihT0_ext),
                             (whhT0, whhT0_ext), (wihT1, wihT1_ext), (whhT1, whhT1_ext),
                             (wd1T, wd1T_ext), (wd2T, wd2T_ext), (bd1, bd1_ext),
                             (bd2rep, bd2rep_ext), (m12, m12_ext), (e0, e0_ext)]:
                nc.sync.dma_start(out=dst, in_=src[:])

            for bg in range(0, NBLK, NI):
                blocks = list(range(bg, min(bg + NI, NBLK)))
                ctx = {}
                for i, b in enumerate(blocks):
                    nb = wp.tile([ND, B], F32R, tag="nb", bufs=2, name=f"nb{b}")
                    nc.sync.dma_start(out=nb, in_=node_t_ext[:, b * B:(b + 1) * B])
                    px = pp.tile([H, B], F32, tag="ps", name=f"px{b}")
                    nc.tensor.matmul(px, wencT_node, nb, start=True, stop=True)
                    enc = wp.tile([H, B], F32R, tag=f"encn{i}", bufs=1, name=f"encn{b}")
                    nc.vector.tensor_copy(enc, px)
                    h1 = [wp.tile([H, B], F32R, tag=f"h1_{i}{p}", bufs=1, name=f"h1_{i}{p}_{b}") for p in range(2)]
                    c1 = [wp.tile([H, B], F32, tag=f"c1_{i}{p}", bufs=1, name=f"c1_{i}{p}_{b}") for p in range(2)]
                    h2 = [wp.tile([H, B], F32R, tag=f"h2_{i}{p}", bufs=1, name=f"h2_{i}{p}_{b}") for p in range(2)]
                    c2 = [wp.tile([H, B], F32R, tag=f"c2_{i}{p}", bufs=1, name=f"c2_{i}{p}_{b}") for p in range(2)]
                    for dst in (h1[0], c1[0], h2[0], c2[0]):
                        nc.vector.tensor_scalar_add(dst, px, e0[:, blkmap[b]:blkmap[b] + 1])
                    yacc = wp.tile([H, 4, THALF, GM], F32, tag=f"yacc{i}", bufs=1, name=f"yacc{b}")
                    stg = [wp.tile([H, 8], F32, tag=f"stg{i}{p}", bufs=1, name=f"stg{i}{p}_{b}")
                           for p in range(2)]
                    ctx[i] = dict(b=b, g=blkmap[b], enc=enc, h1=h1, c1=c1, h2=h2, c2=c2,
                                  yacc=yacc, stg=stg)

                def timestep(i, t, th, k):
                    g = ctx[i]["g"]
                    enc, yacc, stg = ctx[i]["enc"], ctx[i]["yacc"], ctx[i]["stg"]
                    h1, c1, h2, c2 = ctx[i]["h1"], ctx[i]["c1"], ctx[i]["h2"], ctx[i]["c2"]
                    pcur, pnxt = k % 2, 1 - (k % 2)
                    nc.vector.tensor_copy(stg[pcur], m12[:, g, :, DS(t, 1)].squeeze(-1))

                    acts1 = []
                    for c in range(4):
                        pg = pp.tile([H, B], F32, tag="ps", name=f"g1_{c}")
                        nc.tensor.matmul(pg, whhT0[:, c * H:(c + 1) * H], h1[pcur],
                                         start=True, stop=False)
                        nc.tensor.matmul(pg, wihT0[:, c * H:(c + 1) * H], enc,
                                         start=False, stop=True)
                        a = wp.tile([H, B], F32, tag="acts", bufs=14, name=f"a1_{c}")
                        nc.scalar.activation(out=a, in_=pg, func=GATE_FUNCS[c],
                                             bias=stg[pcur][:, c:c + 1], scale=1.0)
                        acts1.append(a)
                    t1 = wp.tile([H, B], F32, tag="prod", bufs=8, name="t1")
                    t2 = wp.tile([H, B], F32, tag="prod", bufs=8, name="t2")
                    nc.vector.tensor_mul(t1, acts1[0], acts1[2])
                    nc.vector.tensor_mul(t2, acts1[1], c1[pcur])
                    nc.vector.tensor_add(c1[pnxt], t1, t2)
                    tc1 = wp.tile([H, B], F32, tag="acts", bufs=14, name="tc1")
                    nc.scalar.activation(out=tc1, in_=c1[pnxt], func=AF.Tanh)
                    nc.vector.tensor_mul(h1[pnxt], acts1[3], tc1)

                    acts2 = []
                    for c in range(4):
                        pg2 = pp.tile([H, B], F32, tag="ps", name=f"g2_{c}")
                        nc.tensor.matmul(pg2, wihT1[0:H - GM, c * H:(c + 1) * H],
                                         h1[pnxt][0:H - GM, :], start=True, stop=False)
                        nc.tensor.matmul(pg2, whhT1[:, c * H:(c + 1) * H], h2[pcur],
                                         start=False, stop=True)
                        a2 = wp.tile([H, B], F32, tag="acts", bufs=14, name=f"a2_{c}")
                        nc.scalar.activation(out=a2, in_=pg2, func=GATE_FUNCS[c],
                                             bias=stg[pcur][:, 4 + c:5 + c], scale=1.0)
                        acts2.append(a2)
                    t3 = wp.tile([H, B], F32, tag="prod", bufs=8, name="t3")
                    t4 = wp.tile([H, B], F32, tag="prod", bufs=8, name="t4")
                    nc.vector.tensor_mul(t3, acts2[0], acts2[2])
                    nc.vector.tensor_mul(t4, acts2[1], c2[pcur])
                    nc.vector.tensor_add(c2[pnxt], t3, t4)
                    tc2 = wp.tile([H, B], F32, tag="acts", bufs=14, name="tc2")
                    nc.scalar.activation(out=tc2, in_=c2[pnxt], func=AF.Tanh)
                    nc.vector.tensor_mul(h2[pnxt], acts2[3], tc2)

                    pd = pp.tile([64, B], F32, tag="ps", name="pd")
                    nc.tensor.matmul(pd, wd1T[:, 0, :], h2[pnxt], start=True, stop=False)
                    nc.tensor.matmul(pd, wd1T[:, 1, :], c2[pnxt], start=False, stop=True)
                    relu = wp.tile([64, B], F32R, tag="relu", bufs=4, name="relu")
                    nc.scalar.activation(out=relu, in_=pd, func=AF.Relu, bias=bd1, scale=1.0)
                    py = pp.tile([H, 16], F32, tag="ps", name="py")
                    for j in range(4):
                        nc.tensor.matmul(py[:, j * 4:(j + 1) * 4],
                                         relu[:, j * H:(j + 1) * H], wd2T,
                                         start=True, stop=True)
                    nc.vector.tensor_add(
                        yacc[:, :, DS(th, 1), :].squeeze(2),
                        py.rearrange("p (j four) -> p j four", four=4)[:, :, 0:GM],
                        bd2rep.unsqueeze(1).broadcast_to([H, 4, GM]))

                for half in range(2):
                    with tc.For_i(0, THALF, UNROLL) as tv:
                        for k in range(UNROLL):
                            for i in range(len(blocks)):
                                timestep(i, tv + k + half * THALF, tv + k, k)
                    for i, b in enumerate(blocks):
                        for j in range(4):
                            nc.sync.dma_start(
                                out=y_ext[b * B + j * H:b * B + (j + 1) * H,
                                          half * THALF:(half + 1) * THALF, :],
                                in_=ctx[i]["yacc"][:, j])

    nc.finalize()
    return nc


_CACHE = {}
_LAST_IN_MAPS = None


def _get_nc(NBLK, NTAB, blkmap):
    key = (NBLK, NTAB, tuple(blkmap))
    if key not in _CACHE:
        import kernel_v3
        _CACHE[key] = kernel_v3.build_nc(NBLK, NTAB, blkmap)
    return _CACHE[key]


def kernel(node, ptr, graph_time_series_behavior, ground_motions,
           W_enc, b_enc, W_ih, W_hh, b_ih, b_hh, W_d1, b_d1, W_d2, b_d2):
    node = np.asarray(node, np.float32)
    ptr = np.asarray(ptr, np.int64)
    lat = np.asarray(graph_time_series_behavior, np.float32)
    gms = np.asarray(ground_motions, np.float32)
    W_enc = np.asarray(W_enc, np.float32); b_enc_a = np.asarray(b_enc, np.float32)
    W_ih = np.asarray(W_ih, np.float32); W_hh = np.asarray(W_hh, np.float32)
    b_ih = np.asarray(b_ih, np.float32); b_hh = np.asarray(b_hh, np.float32)
    W_d1 = np.asarray(W_d1, np.float32); b_d1_a = np.asarray(b_d1, np.float32)
    W_d2 = np.asarray(W_d2, np.float32); b_d2_a = np.asarray(b_d2, np.float32)

    N = node.shape[0]
    BS = lat.shape[0]
    gsizes = np.diff(ptr).astype(np.int64)
    assert gsizes.sum() == N

    gper = (BS + NCORES - 1) // NCORES
    core_graphs = [list(range(c * gper, min((c + 1) * gper, BS))) for c in range(NCORES)]
    NTAB = max(len(cg) for cg in core_graphs)
    core_blkmaps, core_nblk = [], []
    for cg in core_graphs:
        bm = []
        for slot, g in enumerate(cg):
            bm += [slot] * int((gsizes[g] + B - 1) // B)
        core_blkmaps.append(bm)
        core_nblk.append(len(bm))
    NBLK = max(core_nblk) if max(core_nblk) > 0 else 1
    for bm in core_blkmaps:
        bm += [0] * (NBLK - len(bm))
    if all(bm == core_blkmaps[0] for bm in core_blkmaps):
        blkmap = core_blkmaps[0]
        per_block_tabs = False
    else:
        blkmap = list(range(NBLK))
        NTAB = NBLK
        per_block_tabs = True

    NPAD = NBLK * B

    # host-precomputed per-(graph, t) tables (small)
    enc_mix = np.einsum("hk,gtk->gth", W_enc[:, ND:ND + H], lat) \
        + np.einsum("hk,gtk->gth", W_enc[:, ND + H:], gms) + b_enc_a
    m1 = np.einsum("rh,gth->gtr", W_ih[0], enc_mix) + (b_ih[0] + b_hh[0])
    m2 = np.einsum("rk,gtk->gtr", W_ih[1][:, H - GM:], gms) + (b_ih[1] + b_hh[1])
    m12_full = np.concatenate([m1.reshape(BS, T, 4, H), m2.reshape(BS, T, 4, H)], axis=2)
    m12_full = np.ascontiguousarray(m12_full.transpose(3, 0, 2, 1))  # [H, BS, 8, T]
    e0_full = np.ascontiguousarray(enc_mix[:, 0, :].T)               # [H, BS]

    weights_common = dict(
        wencT_node=np.ascontiguousarray(W_enc[:, :ND].T),
        wihT0=np.ascontiguousarray(W_ih[0].T).astype(ml_dtypes.bfloat16),
        whhT0=np.ascontiguousarray(W_hh[0].T).astype(ml_dtypes.bfloat16),
        wihT1=np.ascontiguousarray(W_ih[1].T).astype(ml_dtypes.bfloat16),
        whhT1=np.ascontiguousarray(W_hh[1].T).astype(ml_dtypes.bfloat16),
        wd1T=np.ascontiguousarray(np.stack([W_d1[:, :H].T, W_d1[:, H:].T], axis=1)).astype(ml_dtypes.bfloat16),
        wd2T=np.ascontiguousarray(np.concatenate([W_d2.T, np.zeros((64, 1), np.float32)], 1)).astype(ml_dtypes.bfloat16),
        bd1=b_d1_a.reshape(64, 1),
        bd2rep=np.ascontiguousarray(np.broadcast_to(b_d2_a, (H, GM))),
    )

    in_maps, core_index_maps = [], []
    for c, cg in enumerate(core_graphs):
        node_pad = np.zeros((NPAD, ND), np.float32)
        idx_map = np.full(NPAD, -1, np.int64)
        pos = 0
        for g in cg:
            s, e = int(ptr[g]), int(ptr[g + 1])
            n = e - s
            node_pad[pos:pos + n] = node[s:e]
            idx_map[pos:pos + n] = np.arange(s, e)
            pos += int((n + B - 1) // B) * B
        m12_c = np.zeros((H, NTAB, 8, T), np.float32)
        e0_c = np.zeros((H, NTAB), np.float32)
        if per_block_tabs:
            bi = 0
            for g in cg:
                for _ in range(int((gsizes[g] + B - 1) // B)):
                    m12_c[:, bi] = m12_full[:, g]
                    e0_c[:, bi] = e0_full[:, g]
                    bi += 1
        else:
            for slot, g in enumerate(cg):
                m12_c[:, slot] = m12_full[:, g]
                e0_c[:, slot] = e0_full[:, g]
        in_maps.append(dict(
            node_t=np.ascontiguousarray(node_pad.T),
            m12=m12_c.astype(ml_dtypes.bfloat16),
            e0=e0_c,
            **weights_common,
        ))
        core_index_maps.append(idx_map)

    global _LAST_IN_MAPS
    _LAST_IN_MAPS = in_maps
    nc = _get_nc(NBLK, NTAB, blkmap)
    res = run_bass_kernel_spmd(nc, in_maps, list(range(NCORES)))

    out = np.empty((N, T, GM), np.float32)
    for c in range(NCORES):
        y = res.results[c]["y"]
        m = core_index_maps[c]
        valid = m >= 0
        out[m[valid]] = y[valid]
    return out
